# revision 3
# baseline (speedup 1.0000x reference)
"""AutoCorrelation (factor=3) Trainium2 kernel, 8-core batch-parallel.

Math: the reference's corr tensor [B,L,2047] is only ever used through its
mean over L. That mean collapses to quadratic forms of the Gram matrix
M_b = q_b^T k_b (one [512,512] matmul per batch):
    Zbar[f] = c_f^T M c_f + s_f^T M s_f  + i (c_f^T M s_f - s_f^T M c_f)
mean_value = irfft(Zbar/L, 2047) is a tiny [513]->[2047] cos/sin matmul
(done on host), and the final roll-sum is a circular correlation done
spectrally with constant DFT matrices (all dense matmuls on device).

NEFF1 (per core, batch b): N = k^T q; T1 = N^T C; T2 = N^T S;
    Zr = sum_d (C*T1 + S*T2); Zi = sum_d (C*T2 - S*T1)  -> zout [2,513]
Host: mean_value -> top-20 -> softmax weights w[b], batch-0 shifts ->
    per-batch spectral coefficient vectors a,b,c,d [512].
NEFF2 (per core): Vr/Vi = DFT_L(v) (Nyquist packed into Vi row 0);
    Hr = a*Vr + b*Vi; Hi = c*Vi + d*Vr; out = ICr^T Hr + ICs^T Hi.
"""
import math
import numpy as np

from contextlib import ExitStack
from concourse import bass, mybir, tile, bacc
from concourse.bass_utils import run_bass_kernel_spmd

B, L, D = 8, 1024, 512
NF = L // 2 + 1      # 513
T = 2 * L - 1        # 2047
K = int(3 * math.log(float(L)))  # 20
F32 = mybir.dt.float32

# matmul compute dtype: float32 (safe) or float32r (full-rate fp32 path)
MM_DT = mybir.dt.float32

NCORES = 8
CORE_IDS = list(range(NCORES))

_cache = {}


def _mm(ap):
    """View an fp32 SBUF AP as the matmul compute dtype."""
    if MM_DT == F32:
        return ap
    return ap.bitcast(MM_DT)


# ---------------------------------------------------------------- tables
def _tables():
    if 'tables' in _cache:
        return _cache['tables']
    d = np.arange(D)
    f = np.arange(NF)
    l_idx = np.arange(L)
    p = np.arange(512)

    ang1 = 2 * np.pi * np.outer(d, f) / L              # [512, 513]
    ct = np.cos(ang1)
    st = np.sin(ang1)

    ang2 = 2 * np.pi * np.outer(f, np.arange(T)) / T   # [513, 2047]
    alpha = np.full(NF, 2.0); alpha[0] = 1.0
    C2 = alpha[:, None] * np.cos(ang2) / (T * L)
    S2 = -2.0 * np.sin(ang2) / (T * L); S2[0] = 0.0

    ang = 2 * np.pi * np.outer(l_idx, p) / L           # [1024, 512]
    fc = np.cos(ang)
    fs = -np.sin(ang); fs[:, 0] = (-1.0) ** l_idx      # col0 := Nyquist row
    alp = np.full(512, 2.0); alp[0] = 1.0
    icr = (alp[:, None] * np.cos(ang.T)) / L           # [512, 1024]
    ics = (-2.0 * np.sin(ang.T)) / L
    ics[0, :] = ((-1.0) ** l_idx) / L

    tabs = dict(
        ct=ct.astype(np.float32), st=st.astype(np.float32),
        C2=C2, S2=S2,
        fc=fc.astype(np.float32), fs=fs.astype(np.float32),
        icr=icr.astype(np.float32), ics=ics.astype(np.float32),
    )
    _cache['tables'] = tabs
    return tabs


# ---------------------------------------------------------------- NEFF 1
def build_neff1():
    nc = bacc.Bacc(None, target_bir_lowering=False, debug=False)
    q_d = nc.declare_dram_parameter('q', [L, D], F32, isOutput=False)
    k_d = nc.declare_dram_parameter('k', [L, D], F32, isOutput=False)
    ct_d = nc.declare_dram_parameter('ct', [D, NF], F32, isOutput=False)
    st_d = nc.declare_dram_parameter('st', [D, NF], F32, isOutput=False)
    z_d = nc.declare_dram_parameter('zout', [2, NF], F32, isOutput=True)

    LT, DT = L // 128, D // 128        # 8, 4
    FCH = [(0, 512), (512, 1)]         # f chunks within 513

    with tile.TileContext(nc) as tc, ExitStack() as ctx:
        pool = ctx.enter_context(tc.tile_pool(name='sb', bufs=1))
        psum = ctx.enter_context(
            tc.tile_pool(name='ps', bufs=2, space=bass.MemorySpace.PSUM))
        psz = ctx.enter_context(
            tc.tile_pool(name='psz', bufs=2, space=bass.MemorySpace.PSUM))

        q_sb = pool.tile([128, LT, D], F32)
        k_sb = pool.tile([128, LT, D], F32)
        ct_sb = pool.tile([128, DT, NF], F32)
        st_sb = pool.tile([128, DT, NF], F32)
        for i in range(LT):
            nc.sync.dma_start(q_sb[:, i, :], q_d[i * 128:(i + 1) * 128, :])
            nc.sync.dma_start(k_sb[:, i, :], k_d[i * 128:(i + 1) * 128, :])
        for i in range(DT):
            nc.sync.dma_start(ct_sb[:, i, :], ct_d[i * 128:(i + 1) * 128, :])
            nc.sync.dma_start(st_sb[:, i, :], st_d[i * 128:(i + 1) * 128, :])

        ones = pool.tile([128, 1], F32)
        nc.vector.memset(ones[:], 1.0)

        # N[d2, d1] = sum_l k[l,d2] q[l,d1]
        n_sb = pool.tile([128, DT, D], F32)
        for t2 in range(DT):
            pn = psum.tile([128, D], F32)
            for lt in range(LT):
                nc.tensor.matmul(
                    pn[:],
                    _mm(k_sb[:, lt, t2 * 128:(t2 + 1) * 128]),
                    _mm(q_sb[:, lt, :]),
                    start=(lt == 0), stop=(lt == LT - 1))
            nc.vector.tensor_copy(n_sb[:, t2, :], pn[:])

        # T1[d1, f] = sum_d2 N[d2,d1] ct[d2,f];  T2 with st
        t1_sb = pool.tile([128, DT, NF], F32)
        t2_sb = pool.tile([128, DT, NF], F32)
        for d1t in range(DT):
            for (fo, fw) in FCH:
                pt = psum.tile([128, fw], F32, tag='pt')
                for t2 in range(DT):
                    nc.tensor.matmul(
                        pt[:],
                        _mm(n_sb[:, t2, d1t * 128:(d1t + 1) * 128]),
                        _mm(ct_sb[:, t2, fo:fo + fw]),
                        start=(t2 == 0), stop=(t2 == DT - 1))
                nc.vector.tensor_copy(t1_sb[:, d1t, fo:fo + fw], pt[:])
                pt2 = psum.tile([128, fw], F32, tag='pt')
                for t2 in range(DT):
                    nc.tensor.matmul(
                        pt2[:],
                        _mm(n_sb[:, t2, d1t * 128:(d1t + 1) * 128]),
                        _mm(st_sb[:, t2, fo:fo + fw]),
                        start=(t2 == 0), stop=(t2 == DT - 1))
                nc.vector.tensor_copy(t2_sb[:, d1t, fo:fo + fw], pt2[:])

        # U1 = C*T1 + S*T2 ; U2 = C*T2 - S*T1
        u1_sb = pool.tile([128, DT, NF], F32)
        u2_sb = pool.tile([128, DT, NF], F32)
        for t in range(DT):
            tmp = pool.tile([128, NF], F32, tag='tmp')
            nc.vector.tensor_mul(tmp[:], st_sb[:, t, :], t2_sb[:, t, :])
            nc.vector.tensor_mul(u1_sb[:, t, :], ct_sb[:, t, :], t1_sb[:, t, :])
            nc.vector.tensor_add(u1_sb[:, t, :], u1_sb[:, t, :], tmp[:])
            tmp2 = pool.tile([128, NF], F32, tag='tmp2')
            nc.vector.tensor_mul(tmp2[:], st_sb[:, t, :], t1_sb[:, t, :])
            nc.vector.tensor_mul(u2_sb[:, t, :], ct_sb[:, t, :], t2_sb[:, t, :])
            nc.vector.tensor_sub(u2_sb[:, t, :], u2_sb[:, t, :], tmp2[:])

        # Zr = sum_d U1, Zi = sum_d U2 (partition reduce via ones-matmul)
        z_sb = pool.tile([1, 2, NF], F32)
        for row, u_sb in ((0, u1_sb), (1, u2_sb)):
            for (fo, fw) in FCH:
                pz = psz.tile([1, fw], F32, tag='pz')
                for t in range(DT):
                    nc.tensor.matmul(
                        pz[:], _mm(ones[:]), _mm(u_sb[:, t, fo:fo + fw]),
                        start=(t == 0), stop=(t == DT - 1))
                nc.vector.tensor_copy(z_sb[:, row, fo:fo + fw], pz[:])
        nc.sync.dma_start(z_d[0:1, :], z_sb[:, 0, :])
        nc.sync.dma_start(z_d[1:2, :], z_sb[:, 1, :])

    nc.finalize()
    return nc


# ---------------------------------------------------------------- NEFF 2
def build_neff2():
    nc = bacc.Bacc(None, target_bir_lowering=False, debug=False)
    v_d = nc.declare_dram_parameter('v', [L, D], F32, isOutput=False)
    fc_d = nc.declare_dram_parameter('fc', [L, 512], F32, isOutput=False)
    fs_d = nc.declare_dram_parameter('fs', [L, 512], F32, isOutput=False)
    icr_d = nc.declare_dram_parameter('icr', [512, L], F32, isOutput=False)
    ics_d = nc.declare_dram_parameter('ics', [512, L], F32, isOutput=False)
    a_d = nc.declare_dram_parameter('va', [512, 1], F32, isOutput=False)
    b_d = nc.declare_dram_parameter('vb', [512, 1], F32, isOutput=False)
    c_d = nc.declare_dram_parameter('vc', [512, 1], F32, isOutput=False)
    d_d = nc.declare_dram_parameter('vd', [512, 1], F32, isOutput=False)
    o_d = nc.declare_dram_parameter('out', [L, D], F32, isOutput=True)

    LT, PT = L // 128, 512 // 128      # 8, 4

    with tile.TileContext(nc) as tc, ExitStack() as ctx:
        pool = ctx.enter_context(tc.tile_pool(name='sb', bufs=1))
        outp = ctx.enter_context(tc.tile_pool(name='op', bufs=3))
        psum = ctx.enter_context(
            tc.tile_pool(name='ps', bufs=2, space=bass.MemorySpace.PSUM))
        psum_o = ctx.enter_context(
            tc.tile_pool(name='pso', bufs=2, space=bass.MemorySpace.PSUM))

        v_sb = pool.tile([128, LT, D], F32)
        fc_sb = pool.tile([128, LT, 512], F32)
        fs_sb = pool.tile([128, LT, 512], F32)
        icr_sb = pool.tile([128, PT, L], F32)
        ics_sb = pool.tile([128, PT, L], F32)
        a_sb = pool.tile([128, PT, 1], F32)
        b_sb = pool.tile([128, PT, 1], F32)
        c_sb = pool.tile([128, PT, 1], F32)
        d_sb = pool.tile([128, PT, 1], F32)
        for i in range(LT):
            nc.sync.dma_start(v_sb[:, i, :], v_d[i * 128:(i + 1) * 128, :])
            nc.sync.dma_start(fc_sb[:, i, :], fc_d[i * 128:(i + 1) * 128, :])
            nc.sync.dma_start(fs_sb[:, i, :], fs_d[i * 128:(i + 1) * 128, :])
        for i in range(PT):
            nc.sync.dma_start(icr_sb[:, i, :], icr_d[i * 128:(i + 1) * 128, :])
            nc.sync.dma_start(ics_sb[:, i, :], ics_d[i * 128:(i + 1) * 128, :])
            nc.sync.dma_start(a_sb[:, i, :], a_d[i * 128:(i + 1) * 128, :])
            nc.sync.dma_start(b_sb[:, i, :], b_d[i * 128:(i + 1) * 128, :])
            nc.sync.dma_start(c_sb[:, i, :], c_d[i * 128:(i + 1) * 128, :])
            nc.sync.dma_start(d_sb[:, i, :], d_d[i * 128:(i + 1) * 128, :])

        # forward DFT along l: Vr[p,d], Vi[p,d]
        hr_sb = pool.tile([128, PT, D], F32)
        hi_sb = pool.tile([128, PT, D], F32)
        for pt in range(PT):
            pvr = psum.tile([128, D], F32, tag='pv')
            for lt in range(LT):
                nc.tensor.matmul(
                    pvr[:],
                    _mm(fc_sb[:, lt, pt * 128:(pt + 1) * 128]),
                    _mm(v_sb[:, lt, :]),
                    start=(lt == 0), stop=(lt == LT - 1))
            pvi = psum.tile([128, D], F32, tag='pv')
            for lt in range(LT):
                nc.tensor.matmul(
                    pvi[:],
                    _mm(fs_sb[:, lt, pt * 128:(pt + 1) * 128]),
                    _mm(v_sb[:, lt, :]),
                    start=(lt == 0), stop=(lt == LT - 1))
            # Hr = a*Vr + b*Vi ; Hi = c*Vi + d*Vr  (scalar = per-partition)
            tmp = pool.tile([128, D], F32, tag='htmp')
            nc.vector.tensor_scalar_mul(tmp[:], pvi[:], b_sb[:, pt, :])
            nc.vector.scalar_tensor_tensor(
                hr_sb[:, pt, :], pvr[:], a_sb[:, pt, :], tmp[:],
                mybir.AluOpType.mult, mybir.AluOpType.add)
            tmp2 = pool.tile([128, D], F32, tag='htmp2')
            nc.vector.tensor_scalar_mul(tmp2[:], pvr[:], d_sb[:, pt, :])
            nc.vector.scalar_tensor_tensor(
                hi_sb[:, pt, :], pvi[:], c_sb[:, pt, :], tmp2[:],
                mybir.AluOpType.mult, mybir.AluOpType.add)

        # inverse DFT: out[l,d] = sum_p icr[p,l] Hr[p,d] + ics[p,l] Hi[p,d]
        for lt in range(LT):
            po = psum_o.tile([128, D], F32)
            for pt in range(PT):
                nc.tensor.matmul(
                    po[:],
                    _mm(icr_sb[:, pt, lt * 128:(lt + 1) * 128]),
                    _mm(hr_sb[:, pt, :]),
                    start=(pt == 0), stop=False)
                nc.tensor.matmul(
                    po[:],
                    _mm(ics_sb[:, pt, lt * 128:(lt + 1) * 128]),
                    _mm(hi_sb[:, pt, :]),
                    start=False, stop=(pt == PT - 1))
            o_sb = outp.tile([128, D], F32)
            nc.vector.tensor_copy(o_sb[:], po[:])
            nc.sync.dma_start(o_d[lt * 128:(lt + 1) * 128, :], o_sb[:])

    nc.finalize()
    return nc


# ---------------------------------------------------------------- driver
def _get_graphs():
    if 'nc1' not in _cache:
        _cache['nc1'] = build_neff1()
        _cache['nc2'] = build_neff2()
    return _cache['nc1'], _cache['nc2']


def kernel(queries, keys, values, _trace=False):
    tabs = _tables()
    nc1, nc2 = _get_graphs()
    q = np.ascontiguousarray(np.asarray(queries, np.float32))
    k = np.ascontiguousarray(np.asarray(keys, np.float32))
    v = np.ascontiguousarray(np.asarray(values, np.float32))

    in1 = [{'q': q[b], 'k': k[b], 'ct': tabs['ct'], 'st': tabs['st']}
           for b in range(B)]
    r1 = run_bass_kernel_spmd(nc1, in1, core_ids=CORE_IDS, trace=_trace)
    z = np.stack([r1.results[b]['zout'] for b in range(B)])   # [B, 2, 513]

    mean_value = z[:, 0, :] @ tabs['C2'] + z[:, 1, :] @ tabs['S2']  # [B, T]
    ind = np.argsort(-mean_value, axis=-1, kind='stable')[:, :K]
    val = np.take_along_axis(mean_value, ind, axis=-1)
    e = np.exp(val - val.max(-1, keepdims=True))
    w = e / e.sum(-1, keepdims=True)                          # [B, K]
    shifts = ind[0]                                           # [K]

    f = np.arange(NF)
    ang = 2 * np.pi * np.outer(f, shifts) / L                 # [513, K]
    cosm, sinm = np.cos(ang), np.sin(ang)
    cr = w @ cosm.T                                           # [B, 513]
    ci = -(w @ sinm.T)
    a_v = cr[:, :512].copy()
    b_v = ci[:, :512].copy(); b_v[:, 0] = 0.0
    c_v = cr[:, :512].copy(); c_v[:, 0] = cr[:, 512]
    d_v = -ci[:, :512].copy(); d_v[:, 0] = 0.0

    in2 = [{'v': v[b],
            'fc': tabs['fc'], 'fs': tabs['fs'],
            'icr': tabs['icr'], 'ics': tabs['ics'],
            'va': np.ascontiguousarray(a_v[b].reshape(512, 1), np.float32),
            'vb': np.ascontiguousarray(b_v[b].reshape(512, 1), np.float32),
            'vc': np.ascontiguousarray(c_v[b].reshape(512, 1), np.float32),
            'vd': np.ascontiguousarray(d_v[b].reshape(512, 1), np.float32)}
           for b in range(B)]
    r2 = run_bass_kernel_spmd(nc2, in2, core_ids=CORE_IDS, trace=_trace)
    out = np.stack([r2.results[b]['out'] for b in range(B)])  # [B, L, D]

    kernel._last_exec_ns = (
        (r1.exec_time_ns or 0) + (r2.exec_time_ns or 0)
        if (r1.exec_time_ns or r2.exec_time_ns) else None)
    kernel._last_results = (r1, r2)
    return out.astype(np.float32)
